# revision 6
# baseline (speedup 1.0000x reference)
"""EvidentialUncertaintyDistance Trainium2 kernel.

Reference computation (Nq=2048, Np=256, D=128, H1=128, H2=64):
    qh = q @ W1[:, :D].T            [Nq, H1]
    ph = p @ W1[:, D:].T            [Np, H1]
    h1 = relu(qh[:,None,:] + ph[None,:,:] + b1)         [Nq, Np, H1]
    h2 = relu(einsum('qph,oh->qpo', h1, W2) + b2)       [Nq, Np, H2]
    ev = softplus(einsum('qpo,o->qp', h2, W3[0]) + b3)  [Nq, Np]
    base = max(|q|^2 + |p|^2 - 2 q@p.T, 0)
    out  = base / (1/(ev+1) + 1e-8)  ~= base * (ev + 1)

Sharding: queries split across 8 cores (256 q/core); prototypes + weights
replicated.

Per-core pipeline (p-major):
  - qhT[h,q] / phT[h,p] via float32r matmuls (full-rate, ~1e-4 rel).
  - stage A (DVE): per prototype p, h1T[h, 256q] = relu(qhT + (phT+b1)[:,p])
    as one bf16 4x-mode tensor_scalar op.
  - mm1 (PE): stationary W2T [128h, 64o] bf16; even p's -> psum partitions
    0:64, odd p's -> 64:128 via tile_position=(0,64). psum bank [128, 512]
    holds h2-pre for 4 p's (2 pairs), stacked (o x p-parity) on partitions.
  - stage C (ACT): h2 = relu(psum + b2stack) -> SBUF bf16 (full 128-lane op).
  - mm2 (PE): lhsT = h2 slice [K=128=(o x 2p), M=128 q], rhs = block-diag
    W3 stack [128, 2] -> ev lands DIRECTLY as [128 q, 2 p] columns of a
    persistent psum bank: ev_ps[:, 256*qhalf + p]. No cross-partition
    gather needed anywhere.
  - base (PE): float32r: -2*q@pT (K=128) accumulated with rank-2 update
    (K=2: qn x 1 + 1 x pn) -> base_ps[:, 256*qhalf + p].
  - stage D: conf = softplus(ev+b3)+1 via Exp then Ln(x+1) (one ACT table
    set: natural_log_exp_and_others), out = relu(base) * conf -> DMA.
"""
import numpy as np
import ml_dtypes
from contextlib import ExitStack

import concourse.bass as bass
import concourse.mybir as mybir
import concourse.tile as tile
from concourse import bacc
from concourse.bass_utils import run_bass_kernel_spmd

F32 = mybir.dt.float32
F32R = mybir.dt.float32r
BF16 = mybir.dt.bfloat16
AF = mybir.ActivationFunctionType
OP = mybir.AluOpType

NQ, NP, D, H1, H2 = 2048, 256, 128, 128, 64
NCORES = 8
NQC = NQ // NCORES          # 256 queries per core
NPAIRS = NP // 2            # 128 prototype pairs
NITER = NP // 4             # 64 main-loop iterations (4 p per iter)
MM2_BATCH = 8               # flush mm2 backlog every this many iters

_CACHE = {}


def _r(ap):
    """View an fp32 AP as float32r for full-rate PE streaming."""
    return ap.bitcast(F32R)


def build_bass():
    nc = bacc.Bacc(None, target_bir_lowering=False, debug=False)

    # ---- DRAM I/O (per-core views; all cores run the same program) ----
    qT_d = nc.dram_tensor("qT", [D, NQC], F32R, kind="ExternalInput")
    pT_d = nc.dram_tensor("pT", [D, NP], F32R, kind="ExternalInput")
    pTm2_d = nc.dram_tensor("pTm2", [D, NP], F32R, kind="ExternalInput")
    W1aT_d = nc.dram_tensor("W1aT", [D, H1], F32R, kind="ExternalInput")
    W1bT_d = nc.dram_tensor("W1bT", [D, H1], F32R, kind="ExternalInput")
    W2T_d = nc.dram_tensor("W2T", [H1, H2], BF16, kind="ExternalInput")
    W3s_d = nc.dram_tensor("W3s", [128, 2], BF16, kind="ExternalInput")
    qnA_d = nc.dram_tensor("qnA", [2, NQC], F32R, kind="ExternalInput")
    pn2_d = nc.dram_tensor("pn2", [2, NP], F32R, kind="ExternalInput")
    b1c_d = nc.dram_tensor("b1c", [H1, 1], F32, kind="ExternalInput")
    b2s_d = nc.dram_tensor("b2s", [128, 1], F32, kind="ExternalInput")
    b3c_d = nc.dram_tensor("b3c", [128, 1], F32, kind="ExternalInput")
    out_d = nc.dram_tensor("out", [NQC, NP], F32, kind="ExternalOutput")

    with tile.TileContext(nc) as tc, ExitStack() as ctx:
        consts = ctx.enter_context(tc.tile_pool(name="consts", bufs=1))
        h1pool = ctx.enter_context(tc.tile_pool(name="h1pool", bufs=3))
        h2pool = ctx.enter_context(tc.tile_pool(name="h2pool", bufs=NITER + 2))
        dpool = ctx.enter_context(tc.tile_pool(name="dpool", bufs=2))
        pwork = ctx.enter_context(tc.tile_pool(name="pwork", bufs=1, space="PSUM"))
        pmm = ctx.enter_context(tc.tile_pool(name="pmm", bufs=3, space="PSUM"))
        pres = ctx.enter_context(tc.tile_pool(name="pres", bufs=1, space="PSUM"))

        # ---- load constants ----
        qT_sb = consts.tile([D, NQC], F32R)
        pT_sb = consts.tile([D, NP], F32R)
        pTm2_sb = consts.tile([D, NP], F32R)
        W1aT_sb = consts.tile([D, H1], F32R)
        W1bT_sb = consts.tile([D, H1], F32R)
        W2T_sb = consts.tile([H1, H2], BF16)
        W3s_sb = consts.tile([128, 2], BF16)
        qnA_sb = consts.tile([2, NQC], F32R)
        pn2_sb = consts.tile([2, NP], F32R)
        b1c_sb = consts.tile([H1, 1], F32)
        b2s_sb = consts.tile([128, 1], F32)
        b3c_sb = consts.tile([128, 1], F32)
        one_sb = consts.tile([128, 1], F32)
        for sb_t, d_t in [
            (qT_sb, qT_d), (pT_sb, pT_d), (pTm2_sb, pTm2_d),
            (W1aT_sb, W1aT_d), (W1bT_sb, W1bT_d), (W2T_sb, W2T_d),
            (W3s_sb, W3s_d), (qnA_sb, qnA_d), (pn2_sb, pn2_d),
            (b1c_sb, b1c_d), (b2s_sb, b2s_d), (b3c_sb, b3c_d),
        ]:
            nc.sync.dma_start(out=sb_t, in_=d_t.ap())
        nc.vector.memset(one_sb, 1.0)

        # ---- prolog: qhT, phb ----
        qh_ps = pwork.tile([H1, NQC], F32)
        nc.tensor.matmul(qh_ps, W1aT_sb[:], qT_sb[:], start=True, stop=True)
        qhT_sb = consts.tile([H1, NQC], BF16)
        nc.vector.tensor_copy(qhT_sb, qh_ps)

        ph_ps = pwork.tile([H1, NP], F32)
        nc.tensor.matmul(ph_ps, W1bT_sb[:], pT_sb[:], start=True, stop=True)
        phb_sb = consts.tile([H1, NP], F32)
        nc.vector.tensor_scalar(
            out=phb_sb, in0=ph_ps, scalar1=b1c_sb[:, 0:1], scalar2=None, op0=OP.add)

        # ---- base = qn + pn - 2 q.p  (psum, persistent) ----
        base_ps = pres.tile([128, 2 * NP], F32)
        for h in range(2):
            dst = base_ps[:, h * NP:(h + 1) * NP]
            nc.tensor.matmul(dst, qT_sb[:, h * 128:(h + 1) * 128],
                             pTm2_sb[:], start=True, stop=False)
            nc.tensor.matmul(dst, qnA_sb[:, h * 128:(h + 1) * 128],
                             pn2_sb[:], start=False, stop=True)

        ev_ps = pres.tile([128, 2 * NP], F32)

        # ---- main loop over prototype quads ----
        h2_tiles = []
        pending = []

        def flush_mm2():
            for ii, h2t in pending:
                for local in range(2):          # pair within the quad
                    pp = 2 * ii + local
                    for h in range(2):          # query half
                        nc.tensor.matmul(
                            ev_ps[:, 2 * NP * 0 + NP * h + 2 * pp: NP * h + 2 * pp + 2],
                            h2t[:, 256 * local + 128 * h: 256 * local + 128 * h + 128],
                            W3s_sb[:],
                            start=True, stop=True,
                        )
            pending.clear()

        for i in range(NITER):
            h1t = h1pool.tile([128, 1024], BF16)
            for slot, p in ((0, 4 * i), (1, 4 * i + 2), (2, 4 * i + 1), (3, 4 * i + 3)):
                nc.vector.tensor_scalar(
                    out=h1t[:, 256 * slot: 256 * (slot + 1)],
                    in0=qhT_sb,
                    scalar1=phb_sb[:, p: p + 1], scalar2=0.0,
                    op0=OP.add, op1=OP.max,
                )
            ps = pmm.tile([128, 512], F32)
            nc.tensor.matmul(ps[0:64, :], W2T_sb[:], h1t[:, 0:512],
                             start=True, stop=True)
            nc.tensor.matmul(ps[64:128, :], W2T_sb[:], h1t[:, 512:1024],
                             start=True, stop=True, tile_position=(0, 64))
            h2t = h2pool.tile([128, 512], BF16, tag="h2")
            nc.scalar.activation(out=h2t, in_=ps, func=AF.Relu,
                                 bias=b2s_sb[:, 0:1], scale=1.0)
            pending.append((i, h2t))
            h2_tiles.append(h2t)
            if i % MM2_BATCH == MM2_BATCH - 1:
                flush_mm2()
        flush_mm2()

        # ---- stage D: out = relu(base) * (softplus(ev + b3) + 1) ----
        for h in range(2):
            evs = ev_ps[:, h * NP:(h + 1) * NP]
            bas = base_ps[:, h * NP:(h + 1) * NP]
            t = dpool.tile([128, NP], F32, tag="t")
            nc.scalar.activation(out=t, in_=evs, func=AF.Exp,
                                 bias=b3c_sb[:, 0:1], scale=1.0)
            u = dpool.tile([128, NP], F32, tag="u")
            nc.scalar.activation(out=u, in_=t, func=AF.Ln,
                                 bias=one_sb[:, 0:1], scale=1.0)
            cf = dpool.tile([128, NP], F32, tag="cf")
            nc.vector.tensor_scalar(out=cf, in0=u, scalar1=1.0, scalar2=None,
                                    op0=OP.add)
            rb = dpool.tile([128, NP], F32, tag="rb")
            nc.vector.tensor_scalar(out=rb, in0=bas, scalar1=0.0, scalar2=None,
                                    op0=OP.max)
            ot = dpool.tile([128, NP], F32, tag="ot")
            nc.vector.tensor_mul(ot, rb, cf)
            nc.sync.dma_start(out=out_d.ap()[h * 128:(h + 1) * 128, :], in_=ot)

    nc.compile()
    return nc


def make_in_maps(query_features, prototypes, W1, b1, W2, b2, W3, b3):
    q = np.asarray(query_features, dtype=np.float32)
    p = np.asarray(prototypes, dtype=np.float32)
    W1 = np.asarray(W1, dtype=np.float32)
    W2 = np.asarray(W2, dtype=np.float32)
    W3 = np.asarray(W3, dtype=np.float32)
    b1 = np.asarray(b1, dtype=np.float32)
    b2 = np.asarray(b2, dtype=np.float32)
    b3 = np.asarray(b3, dtype=np.float32)

    pT = np.ascontiguousarray(p.T)                        # [D, NP]
    common = {
        "pT": pT,
        "pTm2": np.ascontiguousarray(-2.0 * pT),
        "W1aT": np.ascontiguousarray(W1[:, :D].T),        # [D, H1]
        "W1bT": np.ascontiguousarray(W1[:, D:].T),
        "W2T": np.ascontiguousarray(W2.T).astype(ml_dtypes.bfloat16),
        "pn2": np.ascontiguousarray(
            np.stack([np.ones(NP, np.float32), (p * p).sum(1)]).astype(np.float32)),
        "b1c": np.ascontiguousarray(b1[:, None]),
        "b2s": np.ascontiguousarray(np.concatenate([b2, b2])[:, None]),
        "b3c": np.full((128, 1), b3[0], np.float32),
    }
    w3s = np.zeros((128, 2), np.float32)
    w3s[0:64, 0] = W3[0]
    w3s[64:128, 1] = W3[0]
    common["W3s"] = w3s.astype(ml_dtypes.bfloat16)

    qn = (q * q).sum(1)                                   # [NQ]
    in_maps = []
    for c in range(NCORES):
        sl = slice(c * NQC, (c + 1) * NQC)
        m = dict(common)
        m["qT"] = np.ascontiguousarray(q[sl].T)           # [D, NQC]
        m["qnA"] = np.ascontiguousarray(
            np.stack([qn[sl], np.ones(NQC, np.float32)]).astype(np.float32))
        in_maps.append(m)
    return in_maps


class Runner:
    """Compile the bass program into a reusable 8-core jitted callable."""

    def __init__(self, nc):
        import jax
        import concourse.mybir as _mybir
        from concourse import bass2jax
        from jax.sharding import Mesh, PartitionSpec
        from jax.experimental.shard_map import shard_map

        bass2jax.install_neuronx_cc_hook()
        self.nc = nc
        partition_name = nc.partition_id_tensor.name if nc.partition_id_tensor else None
        in_names, out_names, out_avals = [], [], []
        for alloc in nc.m.functions[0].allocations:
            if not isinstance(alloc, _mybir.MemoryLocationSet):
                continue
            name = alloc.memorylocations[0].name
            if alloc.kind == "ExternalInput":
                if name != partition_name:
                    in_names.append(name)
            elif alloc.kind == "ExternalOutput":
                out_names.append(name)
                out_avals.append(jax.core.ShapedArray(
                    tuple(alloc.tensor_shape), _mybir.dt.np(alloc.dtype)))
        self.in_names, self.out_names, self.out_avals = in_names, out_names, out_avals
        n_params, n_outs = len(in_names), len(out_names)
        all_names = in_names + out_names
        if partition_name is not None:
            all_names = all_names + [partition_name]

        def _body(*args):
            operands = list(args)
            if partition_name is not None:
                operands.append(bass2jax.partition_id_tensor())
            outs = bass2jax._bass_exec_p.bind(
                *operands,
                out_avals=tuple(out_avals),
                in_names=tuple(all_names),
                out_names=tuple(out_names),
                lowering_input_output_aliases=(),
                sim_require_finite=True,
                sim_require_nnan=True,
                nc=nc,
            )
            return tuple(outs)

        devices = jax.devices()[:NCORES]
        mesh = Mesh(np.asarray(devices), ("core",))
        self.jit = jax.jit(
            shard_map(_body, mesh=mesh,
                      in_specs=(PartitionSpec("core"),) * (n_params + n_outs),
                      out_specs=(PartitionSpec("core"),) * n_outs,
                      check_rep=False),
            keep_unused=True,
        )
        self._jax = jax

    def prep(self, in_maps):
        concat_in = [
            np.concatenate([m[name] for m in in_maps], axis=0)
            for name in self.in_names
        ]
        concat_zeros = [
            np.zeros((NCORES * a.shape[0], *a.shape[1:]), a.dtype)
            for a in self.out_avals
        ]
        return [self._jax.device_put(x) for x in concat_in + concat_zeros]

    def exec(self, args):
        return self._jax.block_until_ready(self.jit(*args))

    def run(self, in_maps):
        outs = self.exec(self.prep(in_maps))
        return [
            {name: np.asarray(outs[i]).reshape(NCORES, *self.out_avals[i].shape)[c]
             for i, name in enumerate(self.out_names)}
            for c in range(NCORES)
        ]


def get_runner():
    if "runner" not in _CACHE:
        _CACHE["runner"] = Runner(build_bass())
    return _CACHE["runner"]


def run(inputs, trace=False, **kw):
    runner = get_runner()
    in_maps = make_in_maps(**inputs)
    results = runner.run(in_maps)
    out = np.concatenate([results[c]["out"] for c in range(NCORES)], axis=0)
    return out.astype(np.float32), results


def kernel(**inputs) -> np.ndarray:
    out, _ = run(inputs)
    return out


if __name__ == "__main__":
    rng = np.random.default_rng(0)
    s1, s2, s3 = 1 / np.sqrt(2 * D), 1 / np.sqrt(H1), 1 / np.sqrt(H2)
    ins = {
        "query_features": rng.standard_normal((NQ, D)).astype(np.float32),
        "prototypes": rng.standard_normal((NP, D)).astype(np.float32),
        "W1": rng.uniform(-s1, s1, (H1, 2 * D)).astype(np.float32),
        "b1": rng.uniform(-s1, s1, (H1,)).astype(np.float32),
        "W2": rng.uniform(-s2, s2, (H2, H1)).astype(np.float32),
        "b2": rng.uniform(-s2, s2, (H2,)).astype(np.float32),
        "W3": rng.uniform(-s3, s3, (1, H2)).astype(np.float32),
        "b3": rng.uniform(-s3, s3, (1,)).astype(np.float32),
    }
    out = kernel(**ins)
    # numpy reference
    q, p = ins["query_features"], ins["prototypes"]
    qh = q @ ins["W1"][:, :D].T
    ph = p @ ins["W1"][:, D:].T
    h1 = np.maximum(qh[:, None, :] + ph[None, :, :] + ins["b1"], 0)
    h2 = np.maximum(h1.reshape(-1, H1) @ ins["W2"].T + ins["b2"], 0)
    z = (h2 @ ins["W3"][0]).reshape(NQ, NP) + ins["b3"][0]
    evd = np.log1p(np.exp(z))
    basem = np.maximum((q * q).sum(1)[:, None] + (p * p).sum(1)[None, :]
                       - 2 * q @ p.T, 0)
    ref = basem / (1.0 / (evd + 1.0) + 1e-8)
    err = np.linalg.norm(out - ref) / np.linalg.norm(ref)
    maxrel = np.max(np.abs(out - ref) / (np.abs(ref) + 1e-6))
    print("norm rel err:", err, " max rel err:", maxrel)


# revision 7
# speedup vs baseline: 4186.1099x; 4186.1099x over previous
"""EvidentialUncertaintyDistance Trainium2 kernel.

Reference computation (Nq=2048, Np=256, D=128, H1=128, H2=64):
    qh = q @ W1[:, :D].T            [Nq, H1]
    ph = p @ W1[:, D:].T            [Np, H1]
    h1 = relu(qh[:,None,:] + ph[None,:,:] + b1)         [Nq, Np, H1]
    h2 = relu(einsum('qph,oh->qpo', h1, W2) + b2)       [Nq, Np, H2]
    ev = softplus(einsum('qpo,o->qp', h2, W3[0]) + b3)  [Nq, Np]
    base = max(|q|^2 + |p|^2 - 2 q@p.T, 0)
    out  = base / (1/(ev+1) + 1e-8)  ~= base * (ev + 1)

Sharding: queries split across 8 cores (256 q/core); prototypes + weights
replicated.

Per-core pipeline (p-major):
  - qhT[h,q] / phT[h,p] via float32r matmuls (full-rate, ~1e-4 rel).
  - stage A (DVE): per prototype p, h1T[h, 256q] = relu(qhT + (phT+b1)[:,p])
    as one bf16 4x-mode tensor_scalar op.
  - mm1 (PE): stationary W2T [128h, 64o] bf16; even p's -> psum partitions
    0:64, odd p's -> 64:128 via tile_position=(0,64). psum bank [128, 512]
    holds h2-pre for 4 p's (2 pairs), stacked (o x p-parity) on partitions.
  - stage C (ACT): h2 = relu(psum + b2stack) -> SBUF bf16 (full 128-lane op).
  - mm2 (PE): lhsT = h2 slice [K=128=(o x 2p), M=128 q], rhs = block-diag
    W3 stack [128, 2] -> ev lands DIRECTLY as [128 q, 2 p] columns of a
    persistent psum bank: ev_ps[:, 256*qhalf + p]. No cross-partition
    gather needed anywhere.
  - base (PE): float32r: -2*q@pT (K=128) accumulated with rank-2 update
    (K=2: qn x 1 + 1 x pn) -> base_ps[:, 256*qhalf + p].
  - stage D: conf = softplus(ev+b3)+1 via Exp then Ln(x+1) (one ACT table
    set: natural_log_exp_and_others), out = relu(base) * conf -> DMA.
"""
import numpy as np
import ml_dtypes
from contextlib import ExitStack

import concourse.bass as bass
import concourse.mybir as mybir
import concourse.tile as tile
from concourse import bacc
from concourse.bass_utils import run_bass_kernel_spmd

F32 = mybir.dt.float32
F32R = mybir.dt.float32r
BF16 = mybir.dt.bfloat16
AF = mybir.ActivationFunctionType
OP = mybir.AluOpType

NQ, NP, D, H1, H2 = 2048, 256, 128, 128, 64
NCORES = 8
NQC = NQ // NCORES          # 256 queries per core
NPAIRS = NP // 2            # 128 prototype pairs
NITER = NP // 4             # 64 main-loop iterations (4 p per iter)
MM2_BATCH = 8               # flush mm2 backlog every this many iters

_CACHE = {}


def _r(ap):
    """View an fp32 AP as float32r for full-rate PE streaming."""
    return ap.bitcast(F32R)


def build_bass():
    nc = bacc.Bacc(None, target_bir_lowering=False, debug=False)

    # ---- DRAM I/O (per-core views; all cores run the same program) ----
    qT_d = nc.dram_tensor("qT", [D, NQC], F32R, kind="ExternalInput")
    pT_d = nc.dram_tensor("pT", [D, NP], F32R, kind="ExternalInput")
    pTm2_d = nc.dram_tensor("pTm2", [D, NP], F32R, kind="ExternalInput")
    W1aT_d = nc.dram_tensor("W1aT", [D, H1], F32R, kind="ExternalInput")
    W1bT_d = nc.dram_tensor("W1bT", [D, H1], F32R, kind="ExternalInput")
    W2T_d = nc.dram_tensor("W2T", [H1, H2], BF16, kind="ExternalInput")
    W3s_d = nc.dram_tensor("W3s", [128, 2], BF16, kind="ExternalInput")
    qnA_d = nc.dram_tensor("qnA", [2, NQC], F32R, kind="ExternalInput")
    pn2_d = nc.dram_tensor("pn2", [2, NP], F32R, kind="ExternalInput")
    b1c_d = nc.dram_tensor("b1c", [H1, 1], F32, kind="ExternalInput")
    b2s_d = nc.dram_tensor("b2s", [128, 1], F32, kind="ExternalInput")
    b3c_d = nc.dram_tensor("b3c", [128, 1], F32, kind="ExternalInput")
    out_d = nc.dram_tensor("out", [NQC, NP], F32, kind="ExternalOutput")

    with tile.TileContext(nc) as tc, ExitStack() as ctx:
        consts = ctx.enter_context(tc.tile_pool(name="consts", bufs=1))
        h1pool = ctx.enter_context(tc.tile_pool(name="h1pool", bufs=3))
        h2pool = ctx.enter_context(tc.tile_pool(name="h2pool", bufs=NITER + 2))
        dpool = ctx.enter_context(tc.tile_pool(name="dpool", bufs=2))
        pwork = ctx.enter_context(tc.tile_pool(name="pwork", bufs=1, space="PSUM"))
        pmm = ctx.enter_context(tc.tile_pool(name="pmm", bufs=3, space="PSUM"))
        pres = ctx.enter_context(tc.tile_pool(name="pres", bufs=1, space="PSUM"))

        # ---- load constants ----
        qT_sb = consts.tile([D, NQC], F32R)
        pT_sb = consts.tile([D, NP], F32R)
        pTm2_sb = consts.tile([D, NP], F32R)
        W1aT_sb = consts.tile([D, H1], F32R)
        W1bT_sb = consts.tile([D, H1], F32R)
        W2T_sb = consts.tile([H1, H2], BF16)
        W3s_sb = consts.tile([128, 2], BF16)
        qnA_sb = consts.tile([2, NQC], F32R)
        pn2_sb = consts.tile([2, NP], F32R)
        b1c_sb = consts.tile([H1, 1], F32)
        b2s_sb = consts.tile([128, 1], F32)
        b3c_sb = consts.tile([128, 1], F32)
        one_sb = consts.tile([128, 1], F32)
        for sb_t, d_t in [
            (qT_sb, qT_d), (pT_sb, pT_d), (pTm2_sb, pTm2_d),
            (W1aT_sb, W1aT_d), (W1bT_sb, W1bT_d), (W2T_sb, W2T_d),
            (W3s_sb, W3s_d), (qnA_sb, qnA_d), (pn2_sb, pn2_d),
            (b1c_sb, b1c_d), (b2s_sb, b2s_d), (b3c_sb, b3c_d),
        ]:
            nc.sync.dma_start(out=sb_t, in_=d_t.ap())
        nc.vector.memset(one_sb, 1.0)

        # ---- prolog: qhT, phb ----
        qh_ps = pwork.tile([H1, NQC], F32)
        nc.tensor.matmul(qh_ps, W1aT_sb[:], qT_sb[:], start=True, stop=True)
        qhT_sb = consts.tile([H1, NQC], BF16)
        nc.vector.tensor_copy(qhT_sb, qh_ps)

        ph_ps = pwork.tile([H1, NP], F32)
        nc.tensor.matmul(ph_ps, W1bT_sb[:], pT_sb[:], start=True, stop=True)
        phb_sb = consts.tile([H1, NP], F32)
        nc.vector.tensor_scalar(
            out=phb_sb, in0=ph_ps, scalar1=b1c_sb[:, 0:1], scalar2=None, op0=OP.add)

        # ---- base = qn + pn - 2 q.p  (psum, persistent) ----
        base_ps = pres.tile([128, 2 * NP], F32)
        for h in range(2):
            dst = base_ps[:, h * NP:(h + 1) * NP]
            nc.tensor.matmul(dst, qT_sb[:, h * 128:(h + 1) * 128],
                             pTm2_sb[:], start=True, stop=False)
            nc.tensor.matmul(dst, qnA_sb[:, h * 128:(h + 1) * 128],
                             pn2_sb[:], start=False, stop=True)

        ev_ps = pres.tile([128, 2 * NP], F32)

        # ---- main loop over prototype quads ----
        h2_tiles = []
        pending = []

        def flush_mm2():
            for ii, h2t in pending:
                for local in range(2):          # pair within the quad
                    pp = 2 * ii + local
                    for h in range(2):          # query half
                        nc.tensor.matmul(
                            ev_ps[:, 2 * NP * 0 + NP * h + 2 * pp: NP * h + 2 * pp + 2],
                            h2t[:, 256 * local + 128 * h: 256 * local + 128 * h + 128],
                            W3s_sb[:],
                            start=True, stop=True,
                        )
            pending.clear()

        for i in range(NITER):
            h1t = h1pool.tile([128, 1024], BF16)
            for slot, p in ((0, 4 * i), (1, 4 * i + 2), (2, 4 * i + 1), (3, 4 * i + 3)):
                nc.vector.tensor_scalar(
                    out=h1t[:, 256 * slot: 256 * (slot + 1)],
                    in0=qhT_sb,
                    scalar1=phb_sb[:, p: p + 1], scalar2=0.0,
                    op0=OP.add, op1=OP.max,
                )
            ps = pmm.tile([128, 512], F32)
            nc.tensor.matmul(ps[0:64, :], W2T_sb[:], h1t[:, 0:512],
                             start=True, stop=True)
            nc.tensor.matmul(ps[64:128, :], W2T_sb[:], h1t[:, 512:1024],
                             start=True, stop=True, tile_position=(0, 64))
            h2t = h2pool.tile([128, 512], BF16, tag="h2")
            nc.scalar.activation(out=h2t, in_=ps, func=AF.Relu,
                                 bias=b2s_sb[:, 0:1], scale=1.0)
            pending.append((i, h2t))
            h2_tiles.append(h2t)
            if i % MM2_BATCH == MM2_BATCH - 1:
                flush_mm2()
        flush_mm2()

        # ---- stage D: out = relu(base) * (softplus(ev + b3) + 1) ----
        for h in range(2):
            evs = ev_ps[:, h * NP:(h + 1) * NP]
            bas = base_ps[:, h * NP:(h + 1) * NP]
            t = dpool.tile([128, NP], F32, tag="t")
            nc.scalar.activation(out=t, in_=evs, func=AF.Exp,
                                 bias=b3c_sb[:, 0:1], scale=1.0)
            u = dpool.tile([128, NP], F32, tag="u")
            nc.scalar.activation(out=u, in_=t, func=AF.Ln,
                                 bias=one_sb[:, 0:1], scale=1.0)
            cf = dpool.tile([128, NP], F32, tag="cf")
            nc.vector.tensor_scalar(out=cf, in0=u, scalar1=1.0, scalar2=None,
                                    op0=OP.add)
            rb = dpool.tile([128, NP], F32, tag="rb")
            nc.vector.tensor_scalar(out=rb, in0=bas, scalar1=0.0, scalar2=None,
                                    op0=OP.max)
            ot = dpool.tile([128, NP], F32, tag="ot")
            nc.vector.tensor_mul(ot, rb, cf)
            nc.sync.dma_start(out=out_d.ap()[h * 128:(h + 1) * 128, :], in_=ot)

    nc.compile()
    return nc


def make_in_maps(query_features, prototypes, W1, b1, W2, b2, W3, b3):
    q = np.asarray(query_features, dtype=np.float32)
    p = np.asarray(prototypes, dtype=np.float32)
    W1 = np.asarray(W1, dtype=np.float32)
    W2 = np.asarray(W2, dtype=np.float32)
    W3 = np.asarray(W3, dtype=np.float32)
    b1 = np.asarray(b1, dtype=np.float32)
    b2 = np.asarray(b2, dtype=np.float32)
    b3 = np.asarray(b3, dtype=np.float32)

    pT = np.ascontiguousarray(p.T)                        # [D, NP]
    common = {
        "pT": pT,
        "pTm2": np.ascontiguousarray(-2.0 * pT),
        "W1aT": np.ascontiguousarray(W1[:, :D].T),        # [D, H1]
        "W1bT": np.ascontiguousarray(W1[:, D:].T),
        "W2T": np.ascontiguousarray(W2.T).astype(ml_dtypes.bfloat16),
        "pn2": np.ascontiguousarray(
            np.stack([np.ones(NP, np.float32), (p * p).sum(1)]).astype(np.float32)),
        "b1c": np.ascontiguousarray(b1[:, None]),
        "b2s": np.ascontiguousarray(np.concatenate([b2, b2])[:, None]),
        "b3c": np.full((128, 1), b3[0], np.float32),
    }
    w3s = np.zeros((128, 2), np.float32)
    w3s[0:64, 0] = W3[0]
    w3s[64:128, 1] = W3[0]
    common["W3s"] = w3s.astype(ml_dtypes.bfloat16)

    qn = (q * q).sum(1)                                   # [NQ]
    in_maps = []
    for c in range(NCORES):
        sl = slice(c * NQC, (c + 1) * NQC)
        m = dict(common)
        m["qT"] = np.ascontiguousarray(q[sl].T)           # [D, NQC]
        m["qnA"] = np.ascontiguousarray(
            np.stack([qn[sl], np.ones(NQC, np.float32)]).astype(np.float32))
        in_maps.append(m)
    return in_maps


class Runner:
    """Compile the bass program into a reusable 8-core jitted callable."""

    def __init__(self, nc):
        import jax
        import concourse.mybir as _mybir
        from concourse import bass2jax
        from jax.sharding import Mesh, PartitionSpec
        from jax.experimental.shard_map import shard_map

        bass2jax.install_neuronx_cc_hook()
        self.nc = nc
        partition_name = nc.partition_id_tensor.name if nc.partition_id_tensor else None
        in_names, out_names, out_avals = [], [], []
        for alloc in nc.m.functions[0].allocations:
            if not isinstance(alloc, _mybir.MemoryLocationSet):
                continue
            name = alloc.memorylocations[0].name
            if alloc.kind == "ExternalInput":
                if name != partition_name:
                    in_names.append(name)
            elif alloc.kind == "ExternalOutput":
                out_names.append(name)
                out_avals.append(jax.core.ShapedArray(
                    tuple(alloc.tensor_shape), _mybir.dt.np(alloc.dtype)))
        self.in_names, self.out_names, self.out_avals = in_names, out_names, out_avals
        n_params, n_outs = len(in_names), len(out_names)
        all_names = in_names + out_names
        if partition_name is not None:
            all_names = all_names + [partition_name]

        def _body(*args):
            operands = list(args)
            if partition_name is not None:
                operands.append(bass2jax.partition_id_tensor())
            outs = bass2jax._bass_exec_p.bind(
                *operands,
                out_avals=tuple(out_avals),
                in_names=tuple(all_names),
                out_names=tuple(out_names),
                lowering_input_output_aliases=(),
                sim_require_finite=True,
                sim_require_nnan=True,
                nc=nc,
            )
            return tuple(outs)

        devices = jax.devices()[:NCORES]
        mesh = Mesh(np.asarray(devices), ("core",))
        self.jit = jax.jit(
            shard_map(_body, mesh=mesh,
                      in_specs=(PartitionSpec("core"),) * (n_params + n_outs),
                      out_specs=(PartitionSpec("core"),) * n_outs,
                      check_rep=False),
            keep_unused=True,
        )
        self._jax = jax

    def make_repeat(self, k):
        """Jitted callable executing the NEFF k times sequentially (ordered
        by BassEffect). Wall(k) - wall(1) isolates device-side time from the
        axon dispatch overhead."""
        import jax
        from jax.sharding import Mesh, PartitionSpec
        from jax.experimental.shard_map import shard_map
        from concourse import bass2jax

        n_params, n_outs = len(self.in_names), len(self.out_names)
        partition_name = (self.nc.partition_id_tensor.name
                          if self.nc.partition_id_tensor else None)
        all_names = self.in_names + self.out_names
        if partition_name is not None:
            all_names = all_names + [partition_name]

        def _bodyk(*args):
            outs = None
            for _ in range(k):
                operands = list(args)
                if partition_name is not None:
                    operands.append(bass2jax.partition_id_tensor())
                outs = bass2jax._bass_exec_p.bind(
                    *operands,
                    out_avals=tuple(self.out_avals),
                    in_names=tuple(all_names),
                    out_names=tuple(self.out_names),
                    lowering_input_output_aliases=(),
                    sim_require_finite=True,
                    sim_require_nnan=True,
                    nc=self.nc,
                )
            return tuple(outs)

        devices = self._jax.devices()[:NCORES]
        mesh = Mesh(np.asarray(devices), ("core",))
        return jax.jit(
            shard_map(_bodyk, mesh=mesh,
                      in_specs=(PartitionSpec("core"),) * (n_params + n_outs),
                      out_specs=(PartitionSpec("core"),) * n_outs,
                      check_rep=False),
            keep_unused=True,
        )

    def prep(self, in_maps):
        concat_in = [
            np.concatenate([m[name] for m in in_maps], axis=0)
            for name in self.in_names
        ]
        concat_zeros = [
            np.zeros((NCORES * a.shape[0], *a.shape[1:]), a.dtype)
            for a in self.out_avals
        ]
        return [self._jax.device_put(x) for x in concat_in + concat_zeros]

    def exec(self, args):
        return self._jax.block_until_ready(self.jit(*args))

    def run(self, in_maps):
        outs = self.exec(self.prep(in_maps))
        return [
            {name: np.asarray(outs[i]).reshape(NCORES, *self.out_avals[i].shape)[c]
             for i, name in enumerate(self.out_names)}
            for c in range(NCORES)
        ]


def get_runner():
    if "runner" not in _CACHE:
        _CACHE["runner"] = Runner(build_bass())
    return _CACHE["runner"]


def run(inputs, trace=False, **kw):
    runner = get_runner()
    in_maps = make_in_maps(**inputs)
    results = runner.run(in_maps)
    out = np.concatenate([results[c]["out"] for c in range(NCORES)], axis=0)
    return out.astype(np.float32), results


def kernel(**inputs) -> np.ndarray:
    out, _ = run(inputs)
    return out


if __name__ == "__main__":
    rng = np.random.default_rng(0)
    s1, s2, s3 = 1 / np.sqrt(2 * D), 1 / np.sqrt(H1), 1 / np.sqrt(H2)
    ins = {
        "query_features": rng.standard_normal((NQ, D)).astype(np.float32),
        "prototypes": rng.standard_normal((NP, D)).astype(np.float32),
        "W1": rng.uniform(-s1, s1, (H1, 2 * D)).astype(np.float32),
        "b1": rng.uniform(-s1, s1, (H1,)).astype(np.float32),
        "W2": rng.uniform(-s2, s2, (H2, H1)).astype(np.float32),
        "b2": rng.uniform(-s2, s2, (H2,)).astype(np.float32),
        "W3": rng.uniform(-s3, s3, (1, H2)).astype(np.float32),
        "b3": rng.uniform(-s3, s3, (1,)).astype(np.float32),
    }
    out = kernel(**ins)
    # numpy reference
    q, p = ins["query_features"], ins["prototypes"]
    qh = q @ ins["W1"][:, :D].T
    ph = p @ ins["W1"][:, D:].T
    h1 = np.maximum(qh[:, None, :] + ph[None, :, :] + ins["b1"], 0)
    h2 = np.maximum(h1.reshape(-1, H1) @ ins["W2"].T + ins["b2"], 0)
    z = (h2 @ ins["W3"][0]).reshape(NQ, NP) + ins["b3"][0]
    evd = np.log1p(np.exp(z))
    basem = np.maximum((q * q).sum(1)[:, None] + (p * p).sum(1)[None, :]
                       - 2 * q @ p.T, 0)
    ref = basem / (1.0 / (evd + 1.0) + 1e-8)
    err = np.linalg.norm(out - ref) / np.linalg.norm(ref)
    maxrel = np.max(np.abs(out - ref) / (np.abs(ref) + 1e-6))
    print("norm rel err:", err, " max rel err:", maxrel)
